# revision 5
# baseline (speedup 1.0000x reference)
"""Trainium2 Bass kernel v6 for nn_CLIP_9560597200942.

Exact jax threefry2x32-20 stream (key 42, 400 samples, partitionable
fold bits = out0^out1, counter j = linear index), E[softmax(mean+eps)].

v5 structure (evolves v4):
  - Both quads (2 x [128,4096] u32, 4 row-tiles each) cipher in
    ANTIPHASE: each round emits quad-wide Pool adds add_a, add_b
    (~6.7us each, the cheapest add width) while DVE shift-xors the
    other quad's slices.  Round cadence is DVE-bound (~18us per 8
    tiles) with Pool ~75% busy -- Pool total drops to ~420us/sample
    vs v4's ~500us.
  - The float pipeline is DEFERRED one sample: fold+pack write pk into
    a dedicated pkstore quad pair at cipher tail, freeing x0/x1 for
    the next sample's cipher; the erfinv/softmax stages then weave
    into the next cipher as DVE filler (small per-round drains).
  - std is stored fp16 (sqrt(var) in [0,1]; 0.05% error, irrelevant
    vs the 2e-2 gate) to fit everything in SBUF.

Sharding: data-parallel over rows, 2048 rows/core on 8 cores.
"""

import numpy as np

import concourse.bass as bass
import concourse.bacc as bacc
import concourse.mybir as mybir
from concourse import tile
from concourse.bass_utils import run_bass_kernel_spmd

A = mybir.AluOpType
AF = mybir.ActivationFunctionType
U32 = mybir.dt.uint32
F32 = mybir.dt.float32
F16 = mybir.dt.float16

# ---------------------------------------------------------------------------
N, C, S = 16384, 512, 400
NCORES = 8
ROWS_PER_CORE = N // NCORES          # 2048
R_PACK = 2
F = R_PACK * C                       # 1024
TILES = ROWS_PER_CORE // (128 * R_PACK)  # 8
QT = 4
QF = QT * F                          # 4096
NQ = TILES // QT                     # 2

ROT = [13, 15, 26, 6, 17, 29, 16, 24]
def _rot(r):
    return ROT[(r - 1) % 4 + (4 if ((r - 1) // 4) % 2 else 0)]

COEF = [1.2543325979649993, -0.0231430622190279, 0.4541265290099754,
        -0.2817079099510704, 0.32774330017955805, -0.17972234858333214,
        0.048544877363447606, -0.006486250945050327, 0.00034478459813376105]
DEG = 8

LO = np.float32(np.nextafter(np.float32(-1.0), np.float32(0.0)))
HILO = np.float32(np.float32(1.0) - LO)

NE = 19


def _keys() -> np.ndarray:
    blob = bytes.fromhex(_KEYS_BLOB)
    return np.frombuffer(blob, dtype=np.uint32).reshape(S, 2)


def _build_ktab(core: int) -> np.ndarray:
    kd = _keys()
    k0 = kd[:, 0]
    k1 = kd[:, 1]
    ks2 = (k0 ^ k1 ^ np.uint32(0x1BD11BDA)).astype(np.uint32)
    ks = [k0, k1, ks2]
    ent = np.zeros((NE, S), dtype=np.uint32)
    for T in range(TILES):
        off = np.uint32((core * (1 << 20) + T * (1 << 17)) & 0xFFFFFFFF)
        ent[T] = (k1 + off).astype(np.uint32)
    ent[8] = k0
    for i in range(1, 6):
        ent[9 + 2 * (i - 1)] = ks[i % 3]
        ent[9 + 2 * (i - 1) + 1] = (ks[(i + 1) % 3] + np.uint32(i)).astype(np.uint32)
    flat = ent.T.reshape(1, S * NE)
    return np.broadcast_to(flat, (128, S * NE)).copy()


def _jp() -> np.ndarray:
    p = np.arange(128, dtype=np.uint32)[:, None]
    f = np.arange(F, dtype=np.uint32)[None, :]
    return (p * np.uint32(F) + f).astype(np.uint32)


def _raw_stt(eng, out, in0, imm, in1, op0, op1):
    return eng.add_instruction(mybir.InstTensorScalarPtr(
        name=eng.bass.get_next_instruction_name(),
        is_scalar_tensor_tensor=True,
        op0=op0, op1=op1,
        ins=[eng.lower_ap(in0),
             mybir.ImmediateValue(dtype=mybir.dt.uint32, value=imm),
             eng.lower_ap(in1)],
        outs=[eng.lower_ap(out)]))


def build_program(nsamples=S, num_devices=NCORES, dyn_loop=True, U=4):
    assert nsamples % U == 0
    nc = bacc.Bacc("TRN2", target_bir_lowering=False, debug=False,
                   num_devices=num_devices)
    mean_d = nc.declare_dram_parameter("mean", [ROWS_PER_CORE, C], F32, isOutput=False)
    var_d = nc.declare_dram_parameter("var", [ROWS_PER_CORE, C], F32, isOutput=False)
    jp_d = nc.declare_dram_parameter("jp", [128, F], U32, isOutput=False)
    ktab_d = nc.declare_dram_parameter("ktab", [128, NE * max(nsamples, S)], U32, isOutput=False)
    out_d = nc.declare_dram_parameter("out", [ROWS_PER_CORE, C], F32, isOutput=True)

    mean_t = mean_d[:].rearrange("(T p r) c -> T p (r c)", p=128, r=R_PACK)
    var_t = var_d[:].rearrange("(T p r) c -> T p (r c)", p=128, r=R_PACK)
    out_t = out_d[:].rearrange("(T p r) c -> T p (r c)", p=128, r=R_PACK)

    with tile.TileContext(nc) as tc:
        with (
            tc.tile_pool(name="persist", bufs=1) as pp,
            tc.tile_pool(name="small", bufs=4) as sp,
        ):
            v, g, a = nc.vector, nc.gpsimd, nc.scalar

            jp = pp.tile([128, F], U32, tag="jp")
            nc.sync.dma_start(jp[:], jp_d[:])
            sqrt_bias = pp.tile([128, 1], F32, tag="sqb")
            g.memset(sqrt_bias[:], 1e-10)

            mean_sb, std_sb, acc_sb = [], [], []
            for T in range(TILES):
                m = pp.tile([128, F], F32, tag=f"mean{T}", name=f"mean{T}")
                sd = pp.tile([128, F], F16, tag=f"std{T}", name=f"std{T}")
                ac = pp.tile([128, F], F32, tag=f"acc{T}", name=f"acc{T}")
                nc.sync.dma_start(m[:], mean_t[T])
                g.memset(ac[:], 0.0)
                mean_sb.append(m); std_sb.append(sd); acc_sb.append(ac)
            # stage var through a work tile, sqrt -> fp16 std
            for T in range(TILES):
                vw = pp.tile([128, F], F32, tag="varw", name="varw", bufs=2)
                nc.sync.dma_start(vw[:], var_t[T])
                a.activation(out=std_sb[T][:], in_=vw[:], func=AF.Sqrt)

            x0q = [pp.tile([128, QF], U32, tag=f"x0q{q}", name=f"x0q{q}")
                   for q in range(NQ)]
            x1q = [pp.tile([128, QF], U32, tag=f"x1q{q}", name=f"x1q{q}")
                   for q in range(NQ)]
            ttp = pp.tile([128, 2 * F], U32, tag="ttp")   # pair-seq b/c scratch
            pkst = [pp.tile([128, QF], U32, tag=f"pk{q}", name=f"pk{q}")
                    for q in range(NQ)]
            fu = pp.tile([128, F], U32, tag="fu")         # float scratch (q)
            fq2 = pp.tile([128, F], U32, tag="fq2")       # float scratch (tq)
            sums = [sp.tile([128, 2], F32, tag=f"sums{t}", bufs=2,
                            name=f"sums{t}") for t in range(TILES)]
            rcps = [sp.tile([128, 2], F32, tag=f"rcp{t}", bufs=2,
                            name=f"rcp{t}") for t in range(TILES)]

            def ktap(stage, e, w=QF):
                return stage[:, e:e + 1].broadcast_to([128, w])

            backlog = []

            def drain(budget_ns=None):
                spent = 0.0
                while backlog and (budget_ns is None or spent < budget_ns):
                    cost, fn = backlog.pop(0)
                    fn()
                    spent += cost

            def push_float():
                """Float pipeline for all 8 tiles, reading pk from pkstore.
                Tile-sequential (shared fu/fq2 scratch)."""
                for T in range(TILES):
                    q, t = T // QT, T % QT
                    pk = pkst[q][:, t * F:(t + 1) * F]
                    ufl = pk.bitcast(F32)
                    tq = fq2[:].bitcast(F32)
                    qq = fu[:].bitcast(F32)
                    backlog.append((0.0, lambda pk=pk, ufl=ufl: a.activation(
                        out=ufl, in_=pk.bitcast(F32), func=AF.Copy, bias=-1.0)))
                    backlog.append((0.0, lambda ufl=ufl: a.activation(
                        out=ufl, in_=ufl, func=AF.Copy, scale=float(HILO),
                        bias=float(LO))))
                    backlog.append((0.0, lambda ufl=ufl, tq=tq: a.activation(
                        out=tq, in_=ufl, func=AF.Square)))
                    backlog.append((0.0, lambda tq=tq: a.activation(
                        out=tq, in_=tq, func=AF.Ln, scale=-1.0, bias=1.0)))
                    backlog.append((0.0, lambda tq=tq: a.activation(
                        out=tq, in_=tq, func=AF.Sqrt, scale=-1.0,
                        bias=sqrt_bias[:])))
                    backlog.append((0.0, lambda tq=tq, qq=qq: a.activation(
                        out=qq, in_=tq, func=AF.Copy, scale=float(COEF[DEG]))))
                    for k in range(DEG - 1, 0, -1):
                        backlog.append((1127.0, lambda qq=qq, tq=tq, k=k:
                            v.scalar_tensor_tensor(out=qq, in0=qq,
                                                   scalar=float(COEF[k]), in1=tq,
                                                   op0=A.add, op1=A.mult)))
                    backlog.append((1127.0, lambda ufl=ufl, T=T: v.tensor_tensor(
                        out=ufl, in0=ufl, in1=std_sb[T][:], op=A.mult)))
                    backlog.append((1127.0, lambda qq=qq, ufl=ufl:
                        v.scalar_tensor_tensor(out=qq, in0=qq,
                                               scalar=float(COEF[0]), in1=ufl,
                                               op0=A.add, op1=A.mult)))
                    backlog.append((1127.0, lambda qq=qq, tq=tq, T=T:
                        v.tensor_tensor(out=tq, in0=qq, in1=mean_sb[T][:],
                                        op=A.add)))
                    pk_u32 = pk
                    for h in range(2):
                        backlog.append((0.0, lambda pk_u32=pk_u32, h=h, T=T:
                            a.activation(
                                out=pk_u32[:, h * C:(h + 1) * C].bitcast(F32),
                                in_=fq2[:, h * C:(h + 1) * C].bitcast(F32),
                                func=AF.Exp,
                                accum_out=sums[T][:, h:h + 1])))
                    backlog.append((200.0, lambda T=T: v.reciprocal(
                        out=rcps[T][:], in_=sums[T][:])))
                    for h in range(2):
                        backlog.append((657.0, lambda pk_u32=pk_u32, h=h, T=T:
                            v.scalar_tensor_tensor(
                                out=acc_sb[T][:, h * C:(h + 1) * C],
                                in0=pk_u32[:, h * C:(h + 1) * C].bitcast(F32),
                                scalar=rcps[T][:, h:h + 1],
                                in1=acc_sb[T][:, h * C:(h + 1) * C],
                                op0=A.mult, op1=A.add)))

            def quad_stts(q, r):
                """b/c shift-xors for quad q, pair-sequenced through ttp."""
                for pr in range(2):
                    ts = (2 * pr, 2 * pr + 1)
                    for i, t in enumerate(ts):
                        _raw_stt(v, ttp[:, i * F:(i + 1) * F],
                                 x1q[q][:, t * F:(t + 1) * F], 32 - r,
                                 x0q[q][:, t * F:(t + 1) * F],
                                 A.logical_shift_right, A.bitwise_xor)
                    for i, t in enumerate(ts):
                        _raw_stt(v, x1q[q][:, t * F:(t + 1) * F],
                                 x1q[q][:, t * F:(t + 1) * F], r,
                                 ttp[:, i * F:(i + 1) * F],
                                 A.logical_shift_left, A.bitwise_xor)

            def sample(stage):
                # init per quad: x1 = jp + (k1+off_T) then x0 = x1 + k0,
                # so quad a's round-1 shift-xors unblock before quad b inits
                for q in range(NQ):
                    for t in range(QT):
                        T = q * QT + t
                        g.tensor_tensor(out=x1q[q][:, t * F:(t + 1) * F],
                                        in0=jp[:],
                                        in1=ktap(stage, T, F), op=A.add)
                    g.tensor_tensor(out=x0q[q][:], in0=x1q[q][:],
                                    in1=ktap(stage, 8), op=A.add)
                drain(28000)
                for rr in range(1, 21):
                    r = _rot(rr)
                    if rr > 1:
                        for q in range(NQ):
                            g.tensor_tensor(out=x0q[q][:], in0=x0q[q][:],
                                            in1=x1q[q][:], op=A.add)
                    for q in range(NQ):
                        quad_stts(q, r)
                    if rr % 4 == 0 and rr < 20:
                        # key injection i4 = rr//4: x0-inj only WARs round
                        # rr's b-reads, so Pool runs it during DVE's stts;
                        # x1-inj waits for c; DVE then drains float while
                        # Pool streams the 4 quad-wide injection adds.
                        i4 = rr // 4
                        for q in range(NQ):
                            g.tensor_tensor(
                                out=x0q[q][:], in0=x0q[q][:],
                                in1=ktap(stage, 9 + 2 * (i4 - 1)),
                                op=A.add)
                        for q in range(NQ):
                            g.tensor_tensor(
                                out=x1q[q][:], in0=x1q[q][:],
                                in1=ktap(stage, 9 + 2 * (i4 - 1) + 1),
                                op=A.add)
                        drain(24000)
                # final key injection (i=5): x0 first (overlaps c of round 20)
                for q in range(NQ):
                    g.tensor_tensor(out=x0q[q][:], in0=x0q[q][:],
                                    in1=ktap(stage, 17), op=A.add)
                for q in range(NQ):
                    g.tensor_tensor(out=x1q[q][:], in0=x1q[q][:],
                                    in1=ktap(stage, 18), op=A.add)
                drain()
                # fold inline into pkstore (frees x0/x1); pack is deferred
                for q in range(NQ):
                    for t in range(QT):
                        v.tensor_tensor(out=pkst[q][:, t * F:(t + 1) * F],
                                        in0=x0q[q][:, t * F:(t + 1) * F],
                                        in1=x1q[q][:, t * F:(t + 1) * F],
                                        op=A.bitwise_xor)
                for q in range(NQ):
                    for t in range(QT):
                        backlog.append((545.0, lambda q=q, t=t: v.tensor_scalar(
                            out=pkst[q][:, t * F:(t + 1) * F],
                            in0=pkst[q][:, t * F:(t + 1) * F],
                            scalar1=9, scalar2=0x3F800000,
                            op0=A.logical_shift_right,
                            op1=A.bitwise_or)))
                push_float()

            def body(idx_expr):
                stage = sp.tile([128, NE], U32, tag="stage", bufs=2 * U)
                nc.sync.dma_start(stage[:], ktab_d[:, bass.ds(idx_expr, NE)])
                return stage

            if dyn_loop:
                with tc.For_i(0, nsamples // U, 1) as it:
                    stages = [body(it * (U * NE) + u * NE) for u in range(U)]
                    for u in range(U):
                        sample(stages[u])
                    drain()
            else:
                stages = [body(i * NE) for i in range(nsamples)]
                for i in range(nsamples):
                    sample(stages[i])
                drain()

            inv = 1.0 / float(nsamples)
            for T in range(TILES):
                o = x0q[0][:, (T % QT) * F:(T % QT + 1) * F]
                a.activation(out=o.bitcast(F32), in_=acc_sb[T][:],
                             func=AF.Copy, scale=inv)
                nc.sync.dma_start(out_t[T], o.bitcast(F32))

    nc.compile()
    return nc


_NC_CACHE = {}


def kernel(mean, var, num_samples):
    mean = np.ascontiguousarray(np.asarray(mean, dtype=np.float32))
    var = np.ascontiguousarray(np.asarray(var, dtype=np.float32))
    ns = int(num_samples)
    assert ns == S, f"kernel is specialized for num_samples={S}, got {ns}"
    assert mean.shape == (N, C) and var.shape == (N, C)

    if "nc" not in _NC_CACHE:
        _NC_CACHE["nc"] = build_program(S)
    nc = _NC_CACHE["nc"]

    jp = _jp()
    in_maps = []
    for d in range(NCORES):
        rs = slice(d * ROWS_PER_CORE, (d + 1) * ROWS_PER_CORE)
        in_maps.append({
            "mean": mean[rs],
            "var": var[rs],
            "jp": jp,
            "ktab": _build_ktab(d),
        })
    res = run_bass_kernel_spmd(nc, in_maps, list(range(NCORES)))
    out = np.empty((N, C), dtype=np.float32)
    for d in range(NCORES):
        out[d * ROWS_PER_CORE:(d + 1) * ROWS_PER_CORE] = res.results[d]["out"]
    return out
